# revision 3
# baseline (speedup 1.0000x reference)
"""Trainium2 Bass kernel for the dense_cnn problem — v2 (4-engine bf16 split).

Computation (per image, C=256, H=W=56):
  t1 = depthwise 5x5 conv(x, w1), pad 2
  t2 = depthwise (3,1) conv(x, w2), pad (1,0)
  t4 = w4 @ unfold(t2, K=7, dil 3, pad 9)   (1x1 mix, C*7 -> C)
  out = t1 * t4

Sharding: pure data parallel, 2 images per core across 8 cores.

v2 design (all data bf16 except PSUM accumulation and the final output):
  - x uploaded once as bf16 in a padded 60x60 layout; all engines read it.
  - t4: 14 bf16 matmul chunks per 448-px tile into PSUM (PE).
  - t1's 25 taps split across engines per tile (counts in UNIT_SPLITS):
      * PE taps: diag-matmul taps accumulate exactly in PSUM (P1),
      * Act taps: products (w*shift(x)) on the Act engine into bf16 temps,
      * Pool taps: tensor_tensor product+add chains on GpSimd with
        pre-broadcast weights (TensorScalarPtr doesn't exist on Pool),
      * DVE merges Act temps + pool partial (bf16 tensor_tensor 2x mode),
        folds P1, then computes the gate out = t1 * t4 in fp32.
  - t2: bf16 tensor_scalar/tensor_tensor chain on DVE (4x/2x modes),
    written into a 74-row padded buffer so t4's unfold is a free SBUF offset.

Measured (8-core axon trn2, reps-loop wall-clock diff): 1.13 ms/invocation
vs 2.17 ms for the fp32r baseline (1.92x); TimelineSim predicts 171 us.
Max rel err vs the fp32 reference: 6.5e-3 (budget 2e-2).
"""

import os
import sys

sys.path.insert(0, "/opt/trn_rl_repo")

import numpy as np
import ml_dtypes

import concourse.bass as bass
import concourse.bacc as bacc
import concourse.mybir as mybir
from concourse.tile import TileContext
from concourse.bass_utils import run_bass_kernel_spmd

# problem constants
N, C, H, W = 16, 256, 56, 56
K, DIL, PAD = 7, 3, 9
HW = H * W                  # 3136
N_CORES = 8
IMGS = N // N_CORES         # 2 images per core
CH = C // 128               # 2 channel halves
PT = 448                    # pixel tile (8 rows of 56)
NPT = HW // PT              # 7 pixel tiles
ROWS_PER_PT = PT // W       # 8
X60 = 60                    # padded x layout: 60 rows x 60 cols
T2ROWS = H + 2 * PAD        # 74 rows in padded t2 buffer
T2LEN = T2ROWS * W          # 4144

# tap split across engines (scan order t = 5*dy + dx), per unit
# (n, oh): unit index u = n*2 + oh.  (pe, act, dve, pool) counts per unit;
# taps are assigned in scan order: PE first, then Act, DVE, Pool.
UNIT_SPLITS = [
    (13, 9, 0, 3),
    (13, 9, 0, 3),
    (13, 9, 0, 3),
    (13, 9, 0, 3),
]
MAX_POOL = 3
MAX_PE = max(s[0] for s in UNIT_SPLITS)


def _unit_taps(u):
    p, a, d, g = UNIT_SPLITS[u]
    assert p + a + d + g == 25
    taps = list(range(25))
    return (taps[:p], taps[p:p + a], taps[p + a:p + a + d],
            taps[p + a + d:])

f32 = mybir.dt.float32
bf16 = mybir.dt.bfloat16
MULT = mybir.AluOpType.mult
ADD = mybir.AluOpType.add

LAST_EXEC_TIME_NS = None
LAST_TRACE_PATH = None

_CACHE = {}


def _build_nc(reps=1):
    """Build the per-core bass program (same NEFF for all 8 cores)."""
    nc = bacc.Bacc()

    npe = MAX_PE
    xp = nc.dram_tensor("xb60", [IMGS, CH, 128, X60 * X60], bf16,
                        kind="ExternalInput")
    w4tp = nc.dram_tensor("w4tp", [K * C, C], bf16, kind="ExternalInput")
    w1d = nc.dram_tensor("w1d", [npe, CH, 128, 128], bf16,
                         kind="ExternalInput")
    wsc = nc.dram_tensor("wsc", [C, 28], f32, kind="ExternalInput")
    wpb = nc.dram_tensor("wpb", [MAX_POOL * CH, 128, PT], bf16,
                         kind="ExternalInput")
    out = nc.dram_tensor("out", [IMGS, C, H, W], f32, kind="ExternalOutput")

    with TileContext(nc) as tc:
        with (
            tc.tile_pool(name="persist", bufs=1) as pp,
            tc.tile_pool(name="t2tmp", bufs=2) as t2pool,
            tc.tile_pool(name="acttmp", bufs=3) as apool,
            tc.tile_pool(name="pooltmp", bufs=3) as gpool,
            tc.tile_pool(name="foldsb", bufs=3) as fpool,
            tc.tile_pool(name="outsb", bufs=2) as opool,
            tc.tile_pool(name="ps1", bufs=4, space="PSUM") as ps1,
            tc.tile_pool(name="ps4", bufs=4, space="PSUM") as ps4,
        ):
            # ---- persistent SBUF ----
            x60 = [[pp.tile([128, X60 * X60], bf16, tag=f"x60_{n}_{h}",
                            name=f"x60_{n}_{h}")
                    for h in range(CH)] for n in range(IMGS)]
            t2p = [[pp.tile([128, T2LEN], bf16, tag=f"t2p_{n}_{h}",
                            name=f"t2p_{n}_{h}")
                    for h in range(CH)] for n in range(IMGS)]
            w4sb = pp.tile([128, 2 * K * C], bf16, tag="w4sb")
            w1dsb = pp.tile([128, npe * CH * 128], bf16, tag="w1dsb")
            wssb = pp.tile([128, CH * 28], f32, tag="wssb")
            wpbsb = pp.tile([128, MAX_POOL * CH * PT], bf16, tag="wpbsb")

            # ---- one-time init: zero the t2p pad rows (never rewritten;
            # memsets on the otherwise-idle Pool engine, no DMA involved) ----
            for n in range(IMGS):
                for h in range(CH):
                    nc.gpsimd.memset(t2p[n][h][:, 0:PAD * W], 0.0)
                    nc.gpsimd.memset(t2p[n][h][:, (PAD + H) * W:T2LEN], 0.0)

            # ---- weight DMAs, spread across issue queues so the startup
            # transfers overlap: scalars + w1 diag on the DVE queue, the big
            # w4 block on the Act queue, x images on the SP queue. ----
            def weight_dmas():
                src = w4tp[:].rearrange("(q p) o -> p q o", p=128)
                nc.scalar.dma_start(
                    out=w4sb[:].rearrange("p (q o) -> p q o", o=C), in_=src)

            # ---- per-image pipeline ----
            import contextlib
            loop_cm = (tc.For_i(0, reps, 1,
                                hint_engines=(mybir.EngineType.PE,
                                              mybir.EngineType.DVE,
                                              mybir.EngineType.SP,
                                              mybir.EngineType.Activation,
                                              mybir.EngineType.Pool))
                       if reps > 1 else contextlib.nullcontext())
            with loop_cm:
                # SP queue order = serial DMA order: tiny scalars first
                # (absorb the bandwidth ramp), then x h0, the w1 diagonals,
                # x h1; the big w4 goes on the Act queue and lands while the
                # PE is still busy with t1 taps.
                nc.sync.dma_start(
                    out=wssb[:].rearrange("p (h s) -> p h s", h=CH),
                    in_=wsc[:].rearrange("(h p) s -> p h s", p=128))
                nc.sync.dma_start(
                    out=wpbsb[:].rearrange("p (s e) -> p s e", s=MAX_POOL * CH),
                    in_=wpb[:].rearrange("s p e -> p s e"))
                nc.sync.dma_start(out=x60[0][0][:], in_=xp[0, 0])
                nc.sync.dma_start(
                    out=w1dsb[:].rearrange("p (t h m) -> p t h m",
                                           t=npe, h=CH),
                    in_=w1d[:].rearrange("t h p m -> p t h m"))
                nc.sync.dma_start(out=x60[0][1][:], in_=xp[0, 1])
                weight_dmas()
                for h in range(CH):
                    nc.sync.dma_start(out=x60[1][h][:], in_=xp[1, h])
                for n in range(IMGS):

                    # t2 (3,1) depthwise on DVE, bf16 fast modes.
                    # t2 row r = sum_j w2[j] * x[r+j-1]; x row r' lives at
                    # xb60 row r'+2, col c at c+2.
                    for h in range(CH):
                        xv = x60[n][h][:].rearrange("p (r c) -> p r c", c=X60)
                        t2int = t2p[n][h][:, PAD * W:(PAD + H) * W]
                        tA = t2pool.tile([128, 2 * HW], bf16, tag="t2t")
                        a0 = tA[:, 0:HW]
                        a1 = tA[:, HW:2 * HW]
                        s0 = h * 28 + 25
                        nc.vector.tensor_scalar_mul(
                            a0.rearrange("p (r c) -> p r c", c=W),
                            xv[:, 1:1 + H, 2:2 + W], wssb[:, s0:s0 + 1])
                        nc.vector.tensor_scalar_mul(
                            a1.rearrange("p (r c) -> p r c", c=W),
                            xv[:, 2:2 + H, 2:2 + W], wssb[:, s0 + 1:s0 + 2])
                        nc.vector.tensor_tensor(
                            out=t2int, in0=a0, in1=a1, op=ADD)
                        nc.vector.tensor_scalar_mul(
                            a0.rearrange("p (r c) -> p r c", c=W),
                            xv[:, 3:3 + H, 2:2 + W], wssb[:, s0 + 2:s0 + 3])
                        nc.vector.tensor_tensor(
                            out=t2int, in0=t2int, in1=a0, op=ADD)

                    for oh in range(CH):
                        pe_taps, act_taps, dve_taps, pool_taps = \
                            _unit_taps(n * CH + oh)
                        xv = x60[n][oh][:].rearrange("p (r c) -> p r c", c=X60)
                        oplane = opool.tile([128, HW], f32)
                        for pt in range(NPT):
                            r0 = pt * ROWS_PER_PT

                            # --- t1 PE taps: diag matmuls into P1 (taps only
                            # need x, so they run before t4 which needs t2) ---
                            pt1 = ps1.tile([128, PT], f32)
                            for i, t in enumerate(pe_taps):
                                ty, tx = divmod(t, 5)
                                rhs = xv[:, r0 + ty:r0 + ty + ROWS_PER_PT,
                                         tx:tx + W]
                                nc.tensor.matmul(
                                    pt1[:],
                                    w1dsb[:, (i * CH + oh) * 128:
                                          (i * CH + oh + 1) * 128],
                                    rhs,
                                    start=(i == 0),
                                    stop=(i == len(pe_taps) - 1))

                            # --- t4: 14-chunk bf16 matmul group (PE) ---
                            pt4 = ps4.tile([128, PT], f32)
                            nq = K * CH
                            for q in range(nq):
                                k, ch = divmod(q, CH)
                                rhs = t2p[n][ch][:, (r0 + k * DIL) * W:
                                                 (r0 + k * DIL) * W + PT]
                                nc.tensor.matmul(
                                    pt4[:],
                                    w4sb[:, q * C + oh * 128:
                                         q * C + oh * 128 + 128],
                                    rhs,
                                    start=(q == 0), stop=(q == nq - 1))

                            # --- t1 Act taps: products into bf16 temps ---
                            na = len(act_taps)
                            atmp = apool.tile([128, 10 * PT], bf16)
                            for i, t in enumerate(act_taps):
                                ty, tx = divmod(t, 5)
                                nc.scalar.mul(
                                    atmp[:, i * PT:(i + 1) * PT]
                                    .rearrange("p (r c) -> p r c", c=W),
                                    xv[:, r0 + ty:r0 + ty + ROWS_PER_PT,
                                       tx:tx + W],
                                    wssb[:, oh * 28 + t:oh * 28 + t + 1])

                            # --- t1 Pool taps: tensor_tensor chain with
                            # pre-broadcast weights (TensorScalarPtr does not
                            # exist on the Pool engine) ---
                            pacc = gpool.tile([128, PT], bf16)
                            ptmp = gpool.tile([128, PT], bf16)
                            for i, t in enumerate(pool_taps):
                                ty, tx = divmod(t, 5)
                                xin = xv[:, r0 + ty:r0 + ty + ROWS_PER_PT,
                                         tx:tx + W]
                                wb = wpbsb[:, (i * CH + oh) * PT:
                                           (i * CH + oh + 1) * PT] \
                                    .rearrange("p (r c) -> p r c", c=W)
                                dst = pacc if i == 0 else ptmp
                                nc.gpsimd.tensor_tensor(
                                    out=dst[:].rearrange(
                                        "p (r c) -> p r c", c=W),
                                    in0=xin, in1=wb, op=MULT)
                                if i > 0:
                                    nc.gpsimd.tensor_tensor(
                                        out=pacc[:], in0=pacc[:],
                                        in1=ptmp[:], op=ADD)

                            # --- DVE: merge temps (bf16 2x), fold P1, gate ---
                            mrg = gpool.tile([128, PT], bf16)
                            nc.vector.tensor_tensor(
                                out=mrg[:], in0=atmp[:, 0:PT],
                                in1=atmp[:, PT:2 * PT], op=ADD)
                            for i in range(2, na):
                                nc.vector.tensor_tensor(
                                    out=mrg[:], in0=mrg[:],
                                    in1=atmp[:, i * PT:(i + 1) * PT], op=ADD)
                            for i, t in enumerate(dve_taps):
                                ty, tx = divmod(t, 5)
                                nc.vector.scalar_tensor_tensor(
                                    out=mrg[:].rearrange(
                                        "p (r c) -> p r c", c=W),
                                    in0=xv[:, r0 + ty:r0 + ty + ROWS_PER_PT,
                                           tx:tx + W],
                                    scalar=wssb[:, oh * 28 + t:
                                                oh * 28 + t + 1],
                                    in1=mrg[:].rearrange(
                                        "p (r c) -> p r c", c=W),
                                    op0=MULT, op1=ADD)
                            nc.vector.tensor_tensor(
                                out=mrg[:], in0=mrg[:], in1=pacc[:], op=ADD)
                            # fold exact PE partial (PSUM fp32): t1f fp32
                            t1f = fpool.tile([128, PT], f32)
                            nc.vector.tensor_tensor(
                                out=t1f[:], in0=mrg[:], in1=pt1[:], op=ADD)
                            # gate into the output plane slice
                            nc.vector.tensor_tensor(
                                out=oplane[:, pt * PT:(pt + 1) * PT],
                                in0=t1f[:], in1=pt4[:], op=MULT)
                            # output DMAs per half-plane, first issued as
                            # soon as its gates are done (shorter tail)
                            oflat = out[n, oh * 128:(oh + 1) * 128] \
                                .rearrange("p r c -> p (r c)")
                            if pt == 3:
                                nc.sync.dma_start(out=oflat[:, 0:HW // 2],
                                                  in_=oplane[:, 0:HW // 2])
                            elif pt == NPT - 1:
                                nc.sync.dma_start(out=oflat[:, HW // 2:HW],
                                                  in_=oplane[:, HW // 2:HW])

    nc.compile()
    return nc


def _prep_inputs(x, w1, w2, w4):
    """Host-side layout prep shared by all cores (weights) + per-core x."""
    x = np.ascontiguousarray(np.asarray(x, dtype=np.float32))
    w1 = np.asarray(w1, dtype=np.float32).reshape(C, 5, 5)
    w2 = np.asarray(w2, dtype=np.float32).reshape(C, 3)
    w4 = np.ascontiguousarray(np.asarray(w4, dtype=np.float32))

    npe = MAX_PE
    # w4 [C, C*K] -> [(k, c), o], bf16
    w4tp = np.ascontiguousarray(
        w4.reshape(C, C, K).transpose(2, 1, 0).reshape(K * C, C)
    ).astype(ml_dtypes.bfloat16)

    # w1 diagonal matrices per (pe-tap-slot, half), bf16
    w1d = np.zeros((npe, CH, 128, 128), dtype=np.float32)
    idx = np.arange(128)
    for i in range(npe):
        ty, tx = divmod(i, 5)
        for h in range(CH):
            w1d[i, h, idx, idx] = w1[h * 128:(h + 1) * 128, ty, tx]
    w1d = w1d.astype(ml_dtypes.bfloat16)

    wsc = np.ascontiguousarray(
        np.concatenate([w1.reshape(C, 25), w2], axis=1))

    # padded per-core x: [IMGS, CH, 128, 60*60] bf16
    xp_all = np.zeros((N, CH, 128, X60, X60), dtype=np.float32)
    xr = x.reshape(N, CH, 128, H, W)
    xp_all[:, :, :, 2:2 + H, 2:2 + W] = xr
    xp_all = xp_all.reshape(N, CH, 128, X60 * X60).astype(ml_dtypes.bfloat16)

    # pool-tap weights pre-broadcast along the pixel tile
    p_, a_, d_, g_ = UNIT_SPLITS[0]
    wpb = np.zeros((MAX_POOL * CH, 128, PT), dtype=np.float32)
    for i in range(g_):
        t = p_ + a_ + d_ + i
        ty, tx = divmod(t, 5)
        for h in range(CH):
            wpb[i * CH + h, :, :] = w1[h * 128:(h + 1) * 128, ty, tx][:, None]
    wpb = wpb.astype(ml_dtypes.bfloat16)

    shared = {"w4tp": w4tp, "w1d": w1d, "wsc": wsc, "wpb": wpb}
    in_maps = []
    for c in range(N_CORES):
        m = dict(shared)
        m["xb60"] = np.ascontiguousarray(xp_all[c * IMGS:(c + 1) * IMGS])
        in_maps.append(m)
    return in_maps


def kernel(x, w1, w2, w4):
    global LAST_EXEC_TIME_NS, LAST_TRACE_PATH
    if "nc" not in _CACHE:
        _CACHE["nc"] = _build_nc()
    nc = _CACHE["nc"]

    in_maps = _prep_inputs(x, w1, w2, w4)
    trace = os.environ.get("BASS_KERNEL_TRACE", "0") == "1"
    try:
        res = run_bass_kernel_spmd(nc, in_maps, core_ids=list(range(N_CORES)),
                                   trace=trace)
    except ModuleNotFoundError:
        res = run_bass_kernel_spmd(nc, in_maps, core_ids=list(range(N_CORES)),
                                   trace=False)
    LAST_EXEC_TIME_NS = res.exec_time_ns
    if res.instructions_and_trace is not None:
        LAST_TRACE_PATH = res.instructions_and_trace[1]
    out = np.concatenate([r["out"] for r in res.results], axis=0)
    return out.astype(np.float32)
